# revision 1
# baseline (speedup 1.0000x reference)
"""Trainium2 Bass kernel for nn_Generator_34127810134219 (gnn_message_passing).

Strategy
--------
The reference relmod builds a [B,N,N] score matrix S = c*x@x^T (diag masked)
and computes wr*(S@U)/N + x.  Algebraically (verified to 4e-7 rel err):

    S@U = c*( x @ (x^T U) - ||x_i||^2 * U_i )

which collapses O(B*N^2*D) work into O(B*N*D^2).  The whole network is then a
memory-light pointwise/matmul pipeline over B*N = 32768 tokens with feature
dims <= 12.

Sharding: data-parallel over batch, 2 batches per core (8 cores).  The only
cross-core coupling is BatchNorm statistics (mean/var per n over batch and
feature dims) - exchanged as tiny [8,512] partial-sum tiles via AllGather
(3x), then reduced locally.  relmod is fully batch-local.

On-chip layout: feature-major, group-packed.  Per core 4096 tokens are split
into 8 groups of 512; group g lives on partitions [16g, 16g+C).  All fc
layers become single 128x512 matmuls with block-diagonal weights (float32r
for full-rate PE).  The per-batch Gram matrix G = x^T U is built with PE
transposes + matmuls; per-group partials are folded per batch as
mask . (Phi^T P_masked Phi) . mask with a fold matrix Phi - PE matmuls only,
no cross-partition vector ops.

All constant patterns (selectors, masks, Phi, block-diagonal weights) are
built on-chip from affine_select primitives + PE matmuls: DMA triggers are
the dominant fixed cost (~0.6us each on the shared HWDGE) so the kernel
issues only ~40 of them.
"""

import numpy as np

import concourse.bass as bass
import concourse.bacc as bacc
import concourse.tile as tile
import concourse.mybir as mybir
from concourse.bass_utils import run_bass_kernel_spmd
from concourse.masks import make_identity

FP32 = mybir.dt.float32
F32R = mybir.dt.float32r
AF = mybir.ActivationFunctionType
OP = mybir.AluOpType

B, N, F = 16, 2048, 3
D2, D4 = 6, 12
NCORES = 8
BPC = B // NCORES          # batches per core
T = BPC * N                # tokens per core
NG = 8                     # groups per core
L = T // NG                # free-dim length (512)
GS = 16                    # partition stride per group
EPS = 1e-5
SAFE_RSTD = False     # Ln+Exp instead of Abs_reciprocal_sqrt
SAFE_SIGMOID = True   # ACT Sigmoid instead of exp+reciprocal_approx

# (name, shape) of every external input except x
PARAM_SPECS = [
    ("fc1_w", (D2, F)), ("fc1_b", (D2,)), ("bn1_g", (N,)), ("bn1_b", (N,)),
    ("fc2_w", (D4, D2)), ("fc2_b", (D4,)), ("bn2_g", (N,)), ("bn2_b", (N,)),
    ("fc3_w", (D4, D4)), ("fc3_b", (D4,)),
    ("u1_w", (D4, D4)), ("u1_b", (D4,)), ("ps1", (1,)), ("ph1", (1,)), ("wr1", (1,)),
    ("u2_w", (D4, D4)), ("u2_b", (D4,)), ("ps2", (1,)), ("ph2", (1,)), ("wr2", (1,)),
    ("u3_w", (D4, D4)), ("u3_b", (D4,)), ("ps3", (1,)), ("ph3", (1,)), ("wr3", (1,)),
    ("u4_w", (D4, D4)), ("u4_b", (D4,)), ("ps4", (1,)), ("ph4", (1,)), ("wr4", (1,)),
    ("fc4_w", (D2, D4)), ("fc4_b", (D2,)), ("bn4_g", (N,)), ("bn4_b", (N,)),
    ("fc5_w", (F, D2)), ("fc5_b", (F,)),
    ("fc6_w", (1, F)), ("fc6_b", (1,)), ("fc7_w", (2, F)), ("fc7_b", (2,)),
]

# weight slot order inside the compact tile (each slot is 16 cols)
W_ORDER = ["fc1_w", "fc2_w", "fc3_w", "u1_w", "u2_w", "u3_w", "u4_w",
           "fc4_w", "fc5_w", "fc67_w"]
B_ORDER = ["fc1_b", "fc2_b", "fc3_b", "u1_b", "u2_b", "u3_b", "u4_b",
           "fc4_b", "fc5_b", "fc67_b"]


def _build(single_core=False):
    nc = bacc.Bacc(
        "TRN2",
        target_bir_lowering=False,
        debug=False,
        enable_asserts=False,
        num_devices=1 if single_core else NCORES,
    )

    x_d = nc.dram_tensor("x", [BPC, N, F], FP32, kind="ExternalInput")
    prm = {
        name: nc.dram_tensor(name, list(shape), FP32, kind="ExternalInput")
        for name, shape in PARAM_SPECS
    }
    out_d = nc.dram_tensor("out", [BPC, N, F], FP32, kind="ExternalOutput")

    with tile.TileContext(nc) as tc:
        with (
            tc.tile_pool(name="consts", bufs=1) as cp,
            tc.tile_pool(name="sb", bufs=1) as sb,
            tc.tile_pool(name="pp", bufs=1, space="PSUM") as pp,
            tc.tile_pool(name="dram", bufs=1, space="DRAM") as dr,
        ):
            _emit(nc, tc, cp, sb, pp, dr, x_d, prm, out_d,
                  single_core=single_core)

    nc.compile()
    return nc


def _emit(nc, tc, cp, sb, pp, dr, x_d, prm, out_d, single_core=False):
    def mmr(out, lhsT, rhs, **kw):
        """float32r matmul: full-rate PE for fp32 bits (reduced mult precision)."""
        nc.tensor.matmul(out, lhsT.bitcast(F32R), rhs.bitcast(F32R), **kw)

    def r(ap):
        """f32r view for producer outputs feeding f32r matmuls (rounds)."""
        return ap.bitcast(F32R)

    eps_t = cp.tile([128, 1], FP32, name="eps_t")
    nc.gpsimd.memset(eps_t[:], EPS)
    # first ACT instruction uses Ln so walrus resolves the
    # natural_log_exp_and_others table set once for the whole kernel
    actwarm = sb.tile([1, 1], FP32, name="actwarm")
    nc.scalar.activation(actwarm[:], eps_t[0:1, :],
                         AF.Ln if SAFE_RSTD else AF.Abs_reciprocal_sqrt)

    # ================= affine-built base selectors (Pool engine) =============
    def affine_sel(t, pattern, cm):
        """t := 1.0 where cm*p + pattern.idx == 0 else 0."""
        nc.gpsimd.memset(t, 0.0)
        nc.gpsimd.affine_select(
            out=t, in_=t, compare_op=OP.not_equal, fill=1.0,
            base=0, pattern=pattern, channel_multiplier=cm)

    # input load first so the network isn't gated on init DMAs
    X = sb.tile([128, L], FP32, name="X")
    nc.gpsimd.memset(X[:], 0.0)
    for g in range(NG):
        b, n0 = g // 4, (g % 4) * L
        eng = nc.sync
        eng.dma_start(X[GS * g:GS * g + F, :],
                      x_d[b, n0:n0 + L, :].rearrange("n c -> c n"))

    # bc8[g, (g',c)] = [g'==g]
    bc8 = cp.tile([NG, 128], FP32, name="bc8")
    affine_sel(bc8[:].rearrange("p (g c) -> p g c", c=GS), [[1, NG], [0, GS]], -1)
    # bc4[j, (g,c)] = [g%4==j]
    bc4 = cp.tile([4, 128], FP32, name="bc4")
    affine_sel(bc4[:].rearrange("p (h j c) -> p h j c", j=4, c=GS),
               [[0, 2], [1, 4], [0, GS]], -1)
    # bcB[b, (g,c)] = [g//4==b]
    bcB = cp.tile([2, 128], FP32, name="bcB")
    affine_sel(bcB[:].rearrange("p (b j c) -> p b j c", j=4, c=GS),
               [[1, 2], [0, 4], [0, GS]], -1)
    # RepSel12[ci', (g,ci)] = [ci==ci'] (ci'<12)
    rsel12 = cp.tile([D4, 128], FP32, name="rsel12")
    affine_sel(rsel12[:].rearrange("p (g c) -> p g c", c=GS), [[0, NG], [1, GS]], -1)
    # RepSel16
    rsel16 = cp.tile([GS, 128], FP32, name="rsel16")
    affine_sel(rsel16[:].rearrange("p (g c) -> p g c", c=GS), [[0, NG], [1, GS]], -1)
    # S8[j, (r,j')] = [j'==j]  (for rank-reduction tiles)
    s8 = cp.tile([8, 64], FP32, name="s8")
    affine_sel(s8[:].rearrange("p (r j) -> p r j", j=8), [[0, 8], [1, 8]], -1)

    ident128 = cp.tile([128, 128], FP32, name="ident128")
    make_identity(nc, ident128[:])
    ones12 = cp.tile([D4, 1], FP32, name="ones12")
    nc.gpsimd.memset(ones12[:], 1.0)
    ones1 = cp.tile([1, 128], FP32, name="ones1")
    nc.gpsimd.memset(ones1[:], 1.0)
    # ================= PE-derived constant tiles =============================
    # mask_diag[(g,c),(g',c')] = [g==g']
    mask_ps = pp.tile([128, 128], FP32, name="mask_ps", tag="b0", padded_shape=[128, L])
    nc.tensor.matmul(mask_ps[:], bc8[:], bc8[:])
    mask_diag = cp.tile([128, 128], FP32, name="mask_diag")
    nc.scalar.activation(mask_diag[:], mask_ps[:], AF.Copy)
    # onesfold [128,4] = bc4^T (needed by the first bn_send pack matmuls)
    of_ps = pp.tile([128, 4], FP32, name="of_ps", tag="b3", padded_shape=[128, L])
    nc.tensor.transpose(of_ps[:], bc4[:], ident128[0:4, 0:4])
    onesfold = cp.tile([128, 4], FP32, name="onesfold")
    nc.scalar.activation(r(onesfold[:]), of_ps[:], AF.Copy)
    # deferred consts (phi/ones_c16/colmask12) are emitted in the bn1
    # AllGather window so they don't sit ahead of fc1 in the PE queue
    phi = cp.tile([128, 128], FP32, name="phi")
    ones_c16 = cp.tile([128, NG], FP32, name="ones_c16")
    colmask12 = cp.tile([128, 1], FP32, name="colmask12")

    def build_deferred_consts():
        crep_ps = pp.tile([128, 128], FP32, name="crep_ps", tag="b1",
                          padded_shape=[128, L])
        nc.tensor.matmul(crep_ps[:], rsel16[:], rsel16[:])
        crep = sb.tile([128, 128], FP32, name="crep")
        nc.scalar.activation(crep[:], crep_ps[:], AF.Copy)
        bmask_ps = pp.tile([128, 128], FP32, name="bmask_ps", tag="b2",
                           padded_shape=[128, L])
        nc.tensor.matmul(bmask_ps[:], bcB[:], bcB[:])
        nc.vector.tensor_tensor(r(phi[:]), bmask_ps[:], crep[:], OP.mult)
        oc_ps = pp.tile([128, NG], FP32, name="oc_ps", tag="b4",
                        padded_shape=[128, L])
        nc.tensor.transpose(oc_ps[:], bc8[:], ident128[0:NG, 0:NG])
        nc.scalar.activation(r(ones_c16[:]), oc_ps[:], AF.Copy)
        cm_ps = pp.tile([128, 1], FP32, name="cm_ps", tag="b5",
                        padded_shape=[128, L])
        nc.tensor.matmul(cm_ps[:], rsel12[:], ones12[:])
        nc.scalar.activation(colmask12[:], cm_ps[:], AF.Copy)
    # f32r-rounded copies of bc4/bc8 (mmr operands must have f32r producers)
    bc4r = cp.tile([4, 128], FP32, name="bc4r")
    nc.vector.tensor_copy(r(bc4r[:]), bc4[:])
    bc8r = cp.tile([NG, 128], FP32, name="bc8r")
    nc.vector.tensor_copy(r(bc8r[:]), bc8[:])
    # rank-reduction tiles [64,8] = S8^T scaled by 1/count
    rr_ps = pp.tile([64, 8], FP32, name="rr_ps", tag="b6", padded_shape=[128, L])
    nc.tensor.transpose(rr_ps[:], s8[:], ident128[0:8, 0:8])
    rr96 = cp.tile([64, 8], FP32, name="rr96")
    nc.scalar.activation(r(rr96[:]), rr_ps[:], AF.Copy, scale=1.0 / 96.0)
    rr192 = cp.tile([64, 8], FP32, name="rr192")
    nc.scalar.activation(r(rr192[:]), rr_ps[:], AF.Copy, scale=1.0 / 192.0)

    # ================= weights / biases ======================================
    # per-weight: Wc[ci,co] -DMA-> [12,16] tile; tp = Wc^T.rsel12 gives the
    # partition-replicated transpose; sp = tp^T.rsel16 spreads along free;
    # mask leaves the block-diagonal lhsT.  build_weight() is emitted at
    # chosen points so init work hides inside collective-wait windows
    # (engines run their streams in order).
    WBD = {}
    _WC = {}

    def load_wc(wname):
        wc = cp.tile([D4, GS], FP32, name=f"wc_{wname}")
        nc.vector.memset(wc[:], 0.0)
        if wname == "fc67_w":
            nc.gpsimd.dma_start(wc[0:F, 0:1],
                                prm["fc6_w"][:, :].rearrange("o i -> i o"))
            nc.gpsimd.dma_start(wc[0:F, 1:3],
                                prm["fc7_w"][:, :].rearrange("o i -> i o"))
        else:
            o, i = prm[wname].shape
            nc.gpsimd.dma_start(wc[0:i, 0:o],
                                prm[wname][:, :].rearrange("o i -> i o"))
        _WC[wname] = wc

    def finish_weight(wname):
        wc = _WC[wname]
        tp = pp.tile([GS, 128], FP32, name=f"wt_{wname}", tag="b6",
                     padded_shape=[128, L])
        nc.tensor.matmul(tp[:], wc[:], rsel12[:])
        ts = sb.tile([GS, 128], FP32, name=f"ws_{wname}", tag="wts")
        nc.scalar.activation(ts[:], tp[:], AF.Copy)
        sp = pp.tile([128, 128], FP32, name=f"wsp_{wname}", tag="b7",
                     padded_shape=[128, L])
        nc.tensor.matmul(sp[:], ts[:], rsel16[:])
        wt = cp.tile([128, 128], FP32, name=f"W_{wname}")
        nc.vector.tensor_tensor(r(wt[:]), sp[:], mask_diag[:], OP.mult)
        WBD[wname] = wt

    load_wc("fc1_w")
    finish_weight("fc1_w")

    BIAS = {}
    _BCV = {}

    def load_bcv(bname):
        bcv = cp.tile([D4, 1], FP32, name=f"bcv_{bname}")
        nc.vector.memset(bcv[:], 0.0)
        if bname == "fc67_b":
            nc.gpsimd.dma_start(bcv[0:1, 0:1],
                                prm["fc6_b"][:].rearrange("(o u) -> o u", u=1))
            nc.gpsimd.dma_start(bcv[1:3, 0:1],
                                prm["fc7_b"][:].rearrange("(o u) -> o u", u=1))
        else:
            cnt = prm[bname].shape[0]
            nc.gpsimd.dma_start(bcv[0:cnt, 0:1],
                                prm[bname][:].rearrange("(o u) -> o u", u=1))
        _BCV[bname] = bcv

    def finish_bias(bname):
        bps = pp.tile([128, 1], FP32, name=f"bps_{bname}", tag="b2",
                      padded_shape=[128, L])
        nc.tensor.matmul(bps[:], rsel12[:], _BCV[bname][:])
        bt = cp.tile([128, 1], FP32, name=f"bias_{bname}")
        nc.scalar.activation(bt[:], bps[:], AF.Copy)
        BIAS[bname] = bt

    load_bcv("fc1_b")
    finish_bias("fc1_b")

    # bn scale/shift as [4, 512]: row j covers n in [512j, 512j+512)
    def bn_vec(name):
        t = cp.tile([4, L], FP32, name=f"v_{name}")
        nc.gpsimd.dma_start(t[:], prm[name][:].rearrange("(j t) -> j t", t=L))
        return t

    bng, bnb = {}, {}

    def load_bn_vecs(k):
        bng[k] = bn_vec(f"{k}_g")
        bnb[k] = bn_vec(f"{k}_b")

    load_bn_vecs("bn1")
    bnb_bc = {}

    def build_bnb_bc(k):
        bps = pp.tile([128, L], FP32, name=f"bnbps_{k}", tag="b3")
        nc.tensor.matmul(bps[:], bc4[:], bnb[k][:])
        bsb = cp.tile([128, L], FP32, name=f"bnbbc_{k}")
        nc.scalar.activation(bsb[:], bps[:], AF.Copy)
        bnb_bc[k] = bsb

    build_bnb_bc("bn1")

    # relmod scale a_r = wr*ps*ph/N as [128,1]
    a_r = []

    def emit_relmod_scales():
      for i in (1, 2, 3, 4):
        pst = sb.tile([1, 1], FP32, name=f"ps_{i}", tag="sc1")
        pht = sb.tile([1, 1], FP32, name=f"ph_{i}", tag="sc2")
        wrt = sb.tile([1, 1], FP32, name=f"wr_{i}", tag="sc3")
        nc.gpsimd.dma_start(pst[:], prm[f"ps{i}"][:].rearrange("(o u) -> o u", u=1))
        nc.gpsimd.dma_start(pht[:], prm[f"ph{i}"][:].rearrange("(o u) -> o u", u=1))
        nc.gpsimd.dma_start(wrt[:], prm[f"wr{i}"][:].rearrange("(o u) -> o u", u=1))
        nc.vector.tensor_tensor(pst[:], pst[:], pht[:], OP.mult)
        nc.vector.tensor_tensor(pst[:], pst[:], wrt[:], OP.mult)
        nc.vector.tensor_scalar_mul(pst[:], pst[:], 1.0 / N)
        pb = pp.tile([128, 1], FP32, name=f"psc_{i}", tag="b3",
                     padded_shape=[128, L])
        nc.tensor.matmul(pb[:], ones1[:], pst[:])
        at = cp.tile([128, 1], FP32, name=f"a_r{i}")
        nc.scalar.activation(at[:], pb[:], AF.Copy)
        a_r.append(at)

    # ================= helpers ===============================================
    def fc(w, src, name, plain=False):
        ps = pp.tile([128, L], FP32, name=f"psfc_{name}", tag="b0")
        if plain:
            nc.tensor.matmul(ps[:], w[:], src[:])
        else:
            mmr(ps[:], w[:], src[:])
        return ps

    def bn_send(h_ps, bias, tag):
        """fc PSUM -> biased hs + partial stats -> AllGather kickoff."""
        hs = sb.tile([128, L], FP32, name=f"hs_{tag}")
        nc.scalar.add(hs[:], h_ps[:], bias)
        sq = sb.tile([128, L], FP32, name=f"sq_{tag}")
        nc.scalar.activation(sq[:], h_ps[:], AF.Square, bias=bias)
        pk_s = pp.tile([4, L], FP32, name=f"pks_{tag}", tag="b1", padded_shape=[128, L])
        pk_q = pp.tile([4, L], FP32, name=f"pkq_{tag}", tag="b2", padded_shape=[128, L])
        nc.tensor.matmul(pk_s[:], onesfold[:], hs[:])
        nc.tensor.matmul(pk_q[:], onesfold[:], sq[:])
        sk_s = sb.tile([4, L], FP32, name=f"sks_{tag}")
        sk_q = sb.tile([4, L], FP32, name=f"skq_{tag}")
        nc.scalar.activation(sk_s[:], pk_s[:], AF.Copy)
        nc.vector.tensor_copy(sk_q[:], pk_q[:])
        cc_in = dr.tile([8, L], FP32, name=f"ccin_{tag}")
        cc_out = dr.tile([64, L], FP32, name=f"ccout_{tag}")
        nc.sync.dma_start(cc_in[0:4, :], sk_s[:])
        nc.scalar.dma_start(cc_in[4:8, :], sk_q[:])
        if single_core:
            # timing-only stand-in for the AllGather (TimelineSim path);
            # 4 serialized DMAs model the ~5us 8-core AllGather latency
            for r in range(4):
                nc.sync.dma_start(cc_out[8 * r:8 * r + 8, :], cc_in[:])
        else:
            nc.gpsimd.collective_compute(
                "AllGather",
                OP.bypass,
                replica_groups=[list(range(NCORES))],
                ins=[cc_in.opt()],
                outs=[cc_out.opt()],
            )
        return hs, cc_out

    def bn_recv(state, key, count_tile, tag):
        """Gathered stats -> bn(h) = a*(h-mean)+beta -> relu."""
        hs, cc_out = state
        gath = sb.tile([64, L], FP32, name=f"gath_{tag}")
        nc.sync.dma_start(gath[:], cc_out[:])
        m_ps = pp.tile([4, L], FP32, name=f"mps_{tag}", tag="b1", padded_shape=[128, L])
        q_ps = pp.tile([4, L], FP32, name=f"qps_{tag}", tag="b2", padded_shape=[128, L])
        nc.tensor.matmul(m_ps[:], count_tile[:, 0:4], gath[:])
        nc.tensor.matmul(q_ps[:], count_tile[:, 4:8], gath[:])
        mean = sb.tile([4, L], FP32, name=f"mean_{tag}")
        nc.scalar.activation(r(mean[:]), m_ps[:], AF.Copy)
        # h - mean (starts as soon as mean is up; off the rstd critical path)
        Mean_bc = pp.tile([128, L], FP32, name=f"Mbc_{tag}", tag="b4")
        mmr(Mean_bc[:], bc4r[:], mean[:])
        t1 = sb.tile([128, L], FP32, name=f"t1_{tag}")
        nc.vector.tensor_tensor(t1[:], hs[:], Mean_bc[:], OP.subtract)
        # a = gamma / sqrt(var+eps); Abs_reciprocal_sqrt is the one-op rstd
        # (ACT Rsqrt proper is banned; var+eps > 0 so abs is a no-op)
        msq = sb.tile([4, L], FP32, name=f"msq_{tag}")
        nc.scalar.activation(msq[:], m_ps[:], AF.Square)
        var = sb.tile([4, L], FP32, name=f"var_{tag}")
        nc.vector.tensor_tensor(var[:], q_ps[:], msq[:], OP.subtract)
        rstd = sb.tile([4, L], FP32, name=f"rstd_{tag}")
        if SAFE_RSTD:
            lv = sb.tile([4, L], FP32, name=f"lv_{tag}")
            nc.scalar.activation(lv[:], var[:], AF.Ln, bias=eps_t[0:4, :])
            nc.scalar.activation(rstd[:], lv[:], AF.Exp, scale=-0.5)
        else:
            nc.scalar.activation(rstd[:], var[:], AF.Abs_reciprocal_sqrt,
                                 bias=eps_t[0:4, :])
        a = sb.tile([4, L], FP32, name=f"a_{tag}")
        nc.vector.tensor_tensor(r(a[:]), rstd[:], bng[key][:], OP.mult)
        A_bc = pp.tile([128, L], FP32, name=f"Abc_{tag}", tag="b3")
        mmr(A_bc[:], bc4r[:], a[:])
        t2 = sb.tile([128, L], FP32, name=f"t2_{tag}")
        nc.vector.tensor_tensor(t2[:], t1[:], A_bc[:], OP.mult)
        t3 = sb.tile([128, L], FP32, name=f"t3_{tag}")
        nc.vector.tensor_tensor(t3[:], t2[:], bnb_bc[key][:], OP.add)
        hn = sb.tile([128, L], FP32, name=f"hn_{tag}")
        nc.vector.tensor_relu(r(hn[:]), t3[:])
        return hn

    def relmod(cur, wu, bu, at, idx):
        psU = pp.tile([128, L], FP32, name=f"psU_{idx}", tag="b0")
        mmr(psU[:], wu[:], cur[:])
        U = sb.tile([128, L], FP32, name=f"U_{idx}", tag="U")
        nc.scalar.activation(U[:], psU[:], AF.Relu, bias=bu)
        # s = sum_c cur^2 per token, broadcast to [128,L]
        sq = sb.tile([128, L], FP32, name=f"rsq_{idx}", tag="rsq")
        nc.scalar.activation(r(sq[:]), cur[:], AF.Square)
        psS = pp.tile([NG, L], FP32, name=f"psS_{idx}", tag="b5", padded_shape=[128, L])
        mmr(psS[:], ones_c16[:], sq[:])
        sS = sb.tile([NG, L], FP32, name=f"sS_{idx}", tag="sS")
        nc.vector.tensor_copy(r(sS[:]), psS[:])
        Sbc = pp.tile([128, L], FP32, name=f"Sbc_{idx}", tag="b3")
        mmr(Sbc[:], bc8r[:], sS[:])
        # transposes of cur and U (4x 128-chunks each)
        pTc = pp.tile([128, 4 * 128], FP32, name=f"pTc_{idx}", tag="b1")
        pTu = pp.tile([128, 4 * 128], FP32, name=f"pTu_{idx}", tag="b2")
        for j in range(4):
            nc.tensor.transpose(
                pTc[:, 128 * j:128 * (j + 1)], cur[:, 128 * j:128 * (j + 1)],
                ident128[:])
            nc.tensor.transpose(
                pTu[:, 128 * j:128 * (j + 1)], U[:, 128 * j:128 * (j + 1)],
                ident128[:])
        curT = sb.tile([128, 4 * 128], FP32, name=f"curT_{idx}", tag="curT")
        UT = sb.tile([128, 4 * 128], FP32, name=f"UT_{idx}", tag="UT")
        nc.scalar.activation(r(curT[:]), pTc[:], AF.Copy)
        nc.vector.tensor_copy(r(UT[:]), pTu[:])
        # P' = sum_t U x cur  (per-group partials on diag blocks)
        psG = pp.tile([128, 128], FP32, name=f"psG_{idx}", tag="b4",
                      padded_shape=[128, L])
        for j in range(4):
            mmr(psG[:], UT[:, 128 * j:128 * (j + 1)],
                curT[:, 128 * j:128 * (j + 1)],
                start=(j == 0), stop=(j == 3))
        Pm = sb.tile([128, 128], FP32, name=f"Pm_{idx}", tag="Pm")
        nc.vector.tensor_tensor(r(Pm[:]), psG[:], mask_diag[:], OP.mult)
        # G_spread = Phi^T (P_m Phi);  P_m = Pm^T
        psM = pp.tile([128, 128], FP32, name=f"psM_{idx}", tag="b5",
                      padded_shape=[128, L])
        mmr(psM[:], Pm[:], phi[:])
        Ms = sb.tile([128, 128], FP32, name=f"Ms_{idx}", tag="Ms")
        nc.scalar.activation(r(Ms[:]), psM[:], AF.Copy)
        psG2 = pp.tile([128, 128], FP32, name=f"psG2_{idx}", tag="b6",
                       padded_shape=[128, L])
        mmr(psG2[:], phi[:], Ms[:])
        Gf = sb.tile([128, 128], FP32, name=f"Gf_{idx}", tag="Gf")
        nc.vector.tensor_tensor(r(Gf[:]), psG2[:], mask_diag[:], OP.mult)
        # xG
        psXG = pp.tile([128, L], FP32, name=f"psXG_{idx}", tag="b6")
        mmr(psXG[:], Gf[:], cur[:])
        # out = (xG - s*U)*a + cur
        sbc_s = sb.tile([128, L], FP32, name=f"sbcs_{idx}", tag="sbcs")
        nc.scalar.activation(sbc_s[:], Sbc[:], AF.Copy)
        w1 = sb.tile([128, L], FP32, name=f"w1_{idx}", tag="w1")
        nc.gpsimd.tensor_tensor(w1[:], sbc_s[:], U[:], OP.mult)
        w2 = sb.tile([128, L], FP32, name=f"w2_{idx}", tag="w2")
        nc.vector.tensor_tensor(w2[:], psXG[:], w1[:], OP.subtract)
        nxt = sb.tile([128, L], FP32, name=f"nxt_{idx}", tag="nxt", bufs=2)
        nc.vector.scalar_tensor_tensor(
            r(nxt[:]), w2[:], at[:], cur[:], OP.mult, OP.add)
        return nxt

    # ================= network ===============================================
    st1 = bn_send(fc(WBD["fc1_w"], X, "1", plain=True), BIAS["fc1_b"][:], "bn1")
    # bn1 AllGather window: queue SWDGE loads + finish fc2/fc3/u1 params
    for w in ("fc2_w", "fc3_w", "u1_w"):
        load_wc(w)
    for b in ("fc2_b", "fc3_b", "u1_b"):
        load_bcv(b)
    load_bn_vecs("bn2")
    build_deferred_consts()
    finish_weight("fc2_w")
    finish_bias("fc2_b")
    finish_weight("fc3_w")
    finish_bias("fc3_b")
    finish_weight("u1_w")
    finish_bias("u1_b")
    build_bnb_bc("bn2")
    h1n = bn_recv(st1, "bn1", rr96, "bn1")
    st2 = bn_send(fc(WBD["fc2_w"], h1n, "2"), BIAS["fc2_b"][:], "bn2")
    # bn2 AllGather window: SWDGE loads first, then u-relmod param finishes
    # (their DMAs land mid-window, before bn2's rank matmuls need the PE)
    emit_relmod_scales()
    for w in ("u2_w", "u3_w", "u4_w"):
        load_wc(w)
    for b in ("u2_b", "u3_b", "u4_b"):
        load_bcv(b)
    for i in (2, 3, 4):
        finish_weight(f"u{i}_w")
        finish_bias(f"u{i}_b")
    h2n = bn_recv(st2, "bn2", rr192, "bn2")
    ps3 = fc(WBD["fc3_w"], h2n, "3")
    enc_r = sb.tile([128, L], FP32, name="enc_r")
    if SAFE_SIGMOID:
        nc.scalar.activation(enc_r[:], ps3[:], AF.Sigmoid, bias=BIAS["fc3_b"][:])
    else:
        # sigmoid(z) = 1/(1+exp(-z)) - keeps ACT on one table set
        b3neg = cp.tile([128, 1], FP32, name="b3neg")
        nc.vector.tensor_scalar_mul(b3neg[:], BIAS["fc3_b"][:], -1.0)
        ex = sb.tile([128, L], FP32, name="ex")
        nc.scalar.activation(ex[:], ps3[:], AF.Exp, scale=-1.0, bias=b3neg[:])
        exp1 = sb.tile([128, L], FP32, name="exp1")
        nc.vector.tensor_scalar_add(exp1[:], ex[:], 1.0)
        rec_scr = sb.tile([128, L], FP32, name="rec_scr")
        nc.vector.reciprocal_approx_accurate(enc_r[:], exp1[:], rec_scr[:])
    # zero the c>=12 garbage rows (sigmoid(0)=0.5) so downstream sums are clean
    enc = sb.tile([128, L], FP32, name="enc")
    nc.vector.tensor_scalar_mul(r(enc[:]), enc_r[:], colmask12[:])

    cur = enc
    for i in range(4):
        cur = relmod(cur, WBD[f"u{i + 1}_w"], BIAS[f"u{i + 1}_b"][:], a_r[i], i)
        if i == 0:
            for w in ("fc4_w", "fc5_w", "fc67_w"):
                load_wc(w)
            for b in ("fc4_b", "fc5_b", "fc67_b"):
                load_bcv(b)
            load_bn_vecs("bn4")
        elif i == 2:
            finish_weight("fc4_w")
            finish_bias("fc4_b")
            build_bnb_bc("bn4")

    st4 = bn_send(fc(WBD["fc4_w"], cur, "4"), BIAS["fc4_b"][:], "bn4")
    finish_weight("fc5_w")
    finish_bias("fc5_b")
    finish_weight("fc67_w")
    finish_bias("fc67_b")
    h4n = bn_recv(st4, "bn4", rr96, "bn4")
    ps5 = fc(WBD["fc5_w"], h4n, "5")
    h5 = sb.tile([128, L], FP32, name="h5")
    nc.scalar.activation(r(h5[:]), ps5[:], AF.Relu, bias=BIAS["fc5_b"][:])
    ps6 = fc(WBD["fc67_w"], h5, "6")
    outs = sb.tile([128, L], FP32, name="outs")
    nc.scalar.add(outs[:], ps6[:], BIAS["fc67_b"][:])

    for g in range(NG):
        b, n0 = g // 4, (g % 4) * L
        eng = nc.sync if g % 2 == 0 else nc.scalar
        eng.dma_start(out_d[b, n0:n0 + L, :].rearrange("n c -> c n"),
                      outs[GS * g:GS * g + F, :])


_PROGRAM = None


def _get_program():
    global _PROGRAM
    if _PROGRAM is None:
        _PROGRAM = _build()
    return _PROGRAM


def run(inputs, trace=False, **kw):
    inputs = {k: np.ascontiguousarray(np.asarray(v, np.float32))
              for k, v in inputs.items()}
    nc = _get_program()
    in_maps = []
    for i in range(NCORES):
        m = {name: inputs[name] for name, _ in PARAM_SPECS}
        m["x"] = np.ascontiguousarray(inputs["x"][BPC * i:BPC * (i + 1)])
        in_maps.append(m)
    last_exc = None
    for attempt in range(3):
        try:
            res = run_bass_kernel_spmd(
                nc, in_maps, core_ids=list(range(NCORES)), trace=trace, **kw)
            break
        except Exception as e:  # transient NRT_EXEC_UNIT_UNRECOVERABLE flakes
            last_exc = e
            import time
            time.sleep(5)
    else:
        raise last_exc
    out = np.concatenate([res.results[i]["out"] for i in range(NCORES)], axis=0)
    return out, res


def kernel(**inputs) -> np.ndarray:
    out, _ = run(inputs)
    return out



# revision 2
# speedup vs baseline: 1.3020x; 1.3020x over previous
"""Trainium2 Bass kernel v2 for nn_Generator_34127810134219 (gnn_message_passing).

Strategy vs v1:
- ALL parameters and constant patterns (block-diagonal weights, masks, phi,
  identity, fold selectors, bn vectors) are packed on the HOST into one
  [128, K] f32 array, loaded as three tiles with overlapped DMAs.
- x is host-packed into the on-chip channel-major layout [128, 512] (one
  contiguous DMA); the output is written token-major to a [128,128] scratch
  (one DMA) and unpacked on host.
- relmod scale a_r = wr*ps*ph/N is folded into the u-weights/biases on host
  (relu(a*z) = a*relu(z), a>0).
- the residual "+cur" of each relmod is folded into the Gram apply by
  accumulating the identity onto the spread Gram matrix (PSUM accumulation),
  so the tail is a single subtract.
- ACT stays on the abs_reciprocal_sqrt table set; the one Tanh (sigmoid via
  0.5+0.5*tanh(z/2)) triggers table swaps that are hidden inside collective
  windows (a dummy Rsq right after the tanh swaps back early).
- bn statistics use AllReduce on raw sums [8,512]; the recv side works on
  sums directly: rstd' = 1/sqrt(q*D - s^2 + eps*D^2), a = (g*D)*rstd',
  shift = a*s/D - beta, saving the mean division.
"""

import numpy as np

import concourse.bass as bass
import concourse.bacc as bacc
import concourse.tile as tile
import concourse.mybir as mybir
from concourse.bass_utils import run_bass_kernel_spmd

FP32 = mybir.dt.float32
F32R = mybir.dt.float32r
AF = mybir.ActivationFunctionType
OP = mybir.AluOpType

B, N, F = 16, 2048, 3
D2, D4 = 6, 12
NCORES = 8
BPC = B // NCORES
NG = 8
L = 512
GS = 16
EPS = 1e-5

# ---- packed-constant column maps (three tiles) -----------------------------
# tile A1 [128, KA1]: needed by fc1/bn1-send
A_BIAS = 0      # 10 cols: fc1, fc2, 0.5*fc3, u1..u4 (a-scaled), fc4, fc5(+1@c3), spare
A_OF = 10       # 4: onesfold[(g,c), j] = [g%4==j]
A_OC16 = 14     # 8: ones_c16[(g,c), g'] = [g==g']
A_COLM = 22     # 1: 0.5*colmask12
A_EPS = 23      # 3: eps*D^2 for bn1, bn2, bn4
A_W1 = 26       # 128: fc1 block-diag
KA1 = 154
# tile A2 [128, KA2]
C_BC4R = 0      # 128 (partitions 0:4): bc4r[j,(g,c)] = [g%4==j]
C_BC8R = 128    # 128 (partitions 0:8): bc8r[j,(g,c)] = [g==j]
C_MASK = 256    # 128: mask_diag
C_ID = 384      # 128: identity
C_W2 = 512      # 8x128: fc2, fc3, u1, u2, u3, u4, fc4, fc5
C_R67 = 1536    # 32: token-major fc6+fc7 rhs (with bias row)
KA2 = 1568
# tile B: phi [128,128]; per-bn vectors [4, 1024] = (g*D | beta)

W_OFF = {"fc2": C_W2, "fc3": C_W2 + 128, "u1": C_W2 + 256,
         "u2": C_W2 + 384, "u3": C_W2 + 512, "u4": C_W2 + 640,
         "fc4": C_W2 + 768, "fc5": C_W2 + 896}
B_IDX = {"fc1": 0, "fc2": 1, "fc3": 2, "u1": 3, "u2": 4, "u3": 5, "u4": 6,
         "fc4": 7, "fc5": 8}
BN_ROW = {"bn1": 0, "bn2": 4, "bn4": 8}
BN_EPS = {"bn1": 0, "bn2": 1, "bn4": 2}


def _build(single_core=False):
    nc = bacc.Bacc(
        "TRN2",
        target_bir_lowering=False,
        debug=False,
        enable_asserts=False,
        num_devices=1 if single_core else NCORES,
    )

    xp_d = nc.dram_tensor("xp", [128, L], FP32, kind="ExternalInput")
    pa1_d = nc.dram_tensor("pa1", [128, KA1], FP32, kind="ExternalInput")
    pa2_d = nc.dram_tensor("pa2", [128, KA2], FP32, kind="ExternalInput")
    pb_d = nc.dram_tensor("pb", [128, 128], FP32, kind="ExternalInput")
    pbn_d = nc.dram_tensor("pbn", [12, 2 * L], FP32, kind="ExternalInput")
    yp_d = nc.dram_tensor("yp", [128, 128], FP32, kind="ExternalOutput")

    with tile.TileContext(nc) as tc:
        with (
            tc.tile_pool(name="consts", bufs=1) as cp,
            tc.tile_pool(name="sb", bufs=1) as sb,
            tc.tile_pool(name="pp", bufs=1, space="PSUM") as pp,
            tc.tile_pool(name="dram", bufs=1, space="DRAM") as dr,
        ):
            _emit(nc, tc, cp, sb, pp, dr, xp_d, pa1_d, pa2_d, pb_d, pbn_d,
                  yp_d, single_core=single_core)

    nc.compile()
    return nc


def _emit(nc, tc, cp, sb, pp, dr, xp_d, pa1_d, pa2_d, pb_d, pbn_d, yp_d,
          single_core=False):
    def mmr(out, lhsT, rhs, **kw):
        nc.tensor.matmul(out, lhsT.bitcast(F32R), rhs.bitcast(F32R), **kw)

    def r(ap):
        return ap.bitcast(F32R)

    # ---- input DMAs (xp most urgent, then A1, A2 on sync; B on SWDGE) ------
    X = sb.tile([128, L], FP32, name="X")
    nc.sync.dma_start(X[:], xp_d[:, :])
    PA1 = cp.tile([128, KA1], FP32, name="PA1")
    nc.sync.dma_start(PA1[:], pa1_d[:, :])
    PA2 = cp.tile([128, KA2], FP32, name="PA2")
    nc.sync.dma_start(PA2[:], pa2_d[:, :])
    PB = cp.tile([128, 128], FP32, name="PB")
    nc.sync.dma_start(PB[:], pb_d[:, :])
    BNV = {}
    for bi, bk in enumerate(("bn1", "bn2", "bn4")):
        BNV[bk] = cp.tile([4, 2 * L], FP32, name=f"bnv_{bk}")
        nc.sync.dma_start(BNV[bk][:], pbn_d[4 * bi:4 * bi + 4, :])

    # ACT table warmup: resolve the abs_reciprocal_sqrt set up front.
    warm = sb.tile([1, 1], FP32, name="warm")
    nc.vector.memset(warm[:], 1.0)
    warmo = sb.tile([1, 1], FP32, name="warmo")
    nc.scalar.activation(warmo[:], warm[:], AF.Abs_reciprocal_sqrt)

    # f32r-rounded copies of every matmul-feeding constant region
    # (walrus requires f32r matmul operands to have f32r producers)
    selr = cp.tile([128, 12], FP32, name="selr")
    nc.vector.tensor_copy(selr[:].bitcast(F32R), PA1[:, A_OF:A_OF + 12])
    PA2r = cp.tile([128, KA2], FP32, name="PA2r")
    nc.vector.tensor_copy(PA2r[:].bitcast(F32R), PA2[:])
    phir = cp.tile([128, 128], FP32, name="phir")
    nc.vector.tensor_copy(phir[:].bitcast(F32R), PB[:])

    def Wv(k):
        if k == "fc1":
            return PA1[:, A_W1:A_W1 + 128]
        return PA2r[:, W_OFF[k]:W_OFF[k] + 128]

    def Bv(k):
        j = B_IDX[k]
        return PA1[:, A_BIAS + j:A_BIAS + j + 1]

    mask = PA2r[:, C_MASK:C_MASK + 128]
    phi = phir[:]
    ident = PA2r[:, C_ID:C_ID + 128]
    bc4r = PA2r[0:4, C_BC4R:C_BC4R + 128]
    bc8r = PA2r[0:NG, C_BC8R:C_BC8R + 128]
    onesfold = selr[:, 0:4]
    oc16 = selr[:, 4:12]
    colm = PA1[:, A_COLM:A_COLM + 1]

    def fc(k, src, plain=False):
        ps = pp.tile([128, L], FP32, name=f"ps_{k}", tag="b0")
        if plain:
            nc.tensor.matmul(ps[:], Wv(k), src[:])
        else:
            mmr(ps[:], Wv(k), src[:])
        return ps

    # ---- batchnorm ---------------------------------------------------------
    def bn_send(h_ps, k, tag):
        hs = sb.tile([128, L], FP32, name=f"hs_{tag}")
        nc.scalar.add(r(hs[:]), h_ps[:], Bv(k))
        sq = sb.tile([128, L], FP32, name=f"sq_{tag}")
        nc.scalar.activation(r(sq[:]), h_ps[:], AF.Square, bias=Bv(k))
        pk_s = pp.tile([4, L], FP32, name=f"pks_{tag}", tag="b1",
                       padded_shape=[128, L])
        pk_q = pp.tile([4, L], FP32, name=f"pkq_{tag}", tag="b2",
                       padded_shape=[128, L])
        mmr(pk_s[:], onesfold, hs[:])
        mmr(pk_q[:], onesfold, sq[:])
        sk = sb.tile([4, 2 * L], FP32, name=f"sk_{tag}")
        nc.scalar.activation(sk[:, 0:L], pk_s[:], AF.Copy)
        nc.vector.tensor_copy(sk[:, L:2 * L], pk_q[:])
        cc_in = dr.tile([8, L], FP32, name=f"ccin_{tag}")
        cc_out = dr.tile([8, L], FP32, name=f"ccout_{tag}")
        nc.sync.dma_start(cc_in[:].rearrange("(r p) t -> p r t", p=4),
                          sk[:].rearrange("p (r t) -> p r t", r=2))
        if single_core:
            # stand-in for the 8-core AllReduce: 3 dummy DMAs + the real
            # cc_out write, serialized on the HWDGE, model ~4-5us latency
            cc_mid = dr.tile([24, L], FP32, name=f"ccmid_{tag}")
            for rr in range(3):
                nc.sync.dma_start(cc_mid[8 * rr:8 * rr + 8, :], cc_in[:])
            nc.sync.dma_start(cc_out[:], cc_in[:])
        else:
            nc.gpsimd.collective_compute(
                "AllReduce",
                OP.add,
                replica_groups=[list(range(NCORES))],
                ins=[cc_in.opt()],
                outs=[cc_out.opt()],
            )
        return hs, cc_out

    def bn_recv(state, k, denom, tag):
        """From AllReduced sums s=rows 0:4, q=rows 4:8 of cc_out:
        rstd' = 1/sqrt(q*D - s^2 + eps*D^2)  (= rstd/D)
        a = (g*D)*rstd'; shift = a*s/D - beta; hn = relu(a*hs - shift)."""
        hs, cc_out = state
        ecol = PA1[0:4, A_EPS + BN_EPS[k]:A_EPS + BN_EPS[k] + 1]
        gath = sb.tile([4, 2 * L], FP32, name=f"gath_{tag}")
        nc.sync.dma_start(gath[:].rearrange("p (r t) -> p r t", r=2),
                          cc_out[:].rearrange("(r p) t -> p r t", p=4))
        msq = sb.tile([4, L], FP32, name=f"msq_{tag}")
        nc.vector.tensor_tensor(msq[:], gath[:, 0:L], gath[:, 0:L], OP.mult)
        varp = sb.tile([4, L], FP32, name=f"varp_{tag}")
        nc.vector.scalar_tensor_tensor(varp[:], gath[:, L:2 * L], denom,
                                       msq[:], OP.mult, OP.subtract)
        rstd = sb.tile([4, L], FP32, name=f"rstd_{tag}")
        nc.scalar.activation(rstd[:], varp[:], AF.Abs_reciprocal_sqrt,
                             bias=ecol)
        a = sb.tile([4, L], FP32, name=f"a_{tag}")
        nc.vector.tensor_tensor(r(a[:]), rstd[:], BNV[k][:, 0:L], OP.mult)
        am = sb.tile([4, L], FP32, name=f"am_{tag}")
        nc.vector.tensor_tensor(am[:], a[:], gath[:, 0:L], OP.mult)
        negd = sb.tile([4, L], FP32, name=f"negd_{tag}")
        nc.vector.scalar_tensor_tensor(r(negd[:]), am[:], 1.0 / denom,
                                       BNV[k][:, L:2 * L],
                                       OP.mult, OP.subtract)
        A_bc = pp.tile([128, L], FP32, name=f"Abc_{tag}", tag="b2")
        mmr(A_bc[:], bc4r, a[:])
        nD_bc = pp.tile([128, L], FP32, name=f"nDbc_{tag}", tag="b3")
        mmr(nD_bc[:], bc4r, negd[:])
        t2 = sb.tile([128, L], FP32, name=f"t2_{tag}")
        nc.vector.tensor_tensor(t2[:], hs[:], A_bc[:], OP.mult)
        t3 = sb.tile([128, L], FP32, name=f"t3_{tag}")
        nc.vector.tensor_tensor(t3[:], t2[:], nD_bc[:], OP.subtract)
        hn = sb.tile([128, L], FP32, name=f"hn_{tag}")
        nc.vector.tensor_relu(r(hn[:]), t3[:])
        return hn

    # ---- relation module ---------------------------------------------------
    def relmod(cur, k, idx):
        # PE: psU, pTc, pTu, psS, psG, Sbc, psM, psG2(+I), psXG
        psU = pp.tile([128, L], FP32, name=f"psU_{idx}", tag="b0")
        mmr(psU[:], Wv(k), cur[:])
        pTc = pp.tile([128, 4 * 128], FP32, name=f"pTc_{idx}", tag="b1")
        for j in range(4):
            nc.tensor.transpose(pTc[:, 128 * j:128 * (j + 1)],
                                cur[:, 128 * j:128 * (j + 1)], ident)
        U = sb.tile([128, L], FP32, name=f"U_{idx}", tag="U")
        nc.scalar.activation(r(U[:]), psU[:], AF.Relu, bias=Bv(k))
        sq = sb.tile([128, L], FP32, name=f"rsq_{idx}", tag="rsq")
        nc.scalar.activation(r(sq[:]), cur[:], AF.Square)
        curT = sb.tile([128, 4 * 128], FP32, name=f"curT_{idx}", tag="curT")
        nc.vector.tensor_copy(r(curT[:]), pTc[:])
        pTu = pp.tile([128, 4 * 128], FP32, name=f"pTu_{idx}", tag="b2")
        for j in range(4):
            nc.tensor.transpose(pTu[:, 128 * j:128 * (j + 1)],
                                U[:, 128 * j:128 * (j + 1)], ident)
        UT = sb.tile([128, 4 * 128], FP32, name=f"UT_{idx}", tag="UT")
        nc.vector.tensor_copy(r(UT[:]), pTu[:])
        psS = pp.tile([NG, L], FP32, name=f"psS_{idx}", tag="b3",
                      padded_shape=[128, L])
        mmr(psS[:], oc16, sq[:])
        psG = pp.tile([128, 128], FP32, name=f"psG_{idx}", tag="b4",
                      padded_shape=[128, L])
        for j in range(4):
            mmr(psG[:], UT[:, 128 * j:128 * (j + 1)],
                curT[:, 128 * j:128 * (j + 1)],
                start=(j == 0), stop=(j == 3))
        sS = sb.tile([NG, L], FP32, name=f"sS_{idx}", tag="sS")
        nc.vector.tensor_copy(r(sS[:]), psS[:])
        Sbc = pp.tile([128, L], FP32, name=f"Sbc_{idx}", tag="b1")
        mmr(Sbc[:], bc8r, sS[:])
        sbc_s = sb.tile([128, L], FP32, name=f"sbcs_{idx}", tag="sbcs")
        nc.scalar.activation(sbc_s[:], Sbc[:], AF.Copy)
        Pm = sb.tile([128, 128], FP32, name=f"Pm_{idx}", tag="Pm")
        nc.vector.tensor_tensor(r(Pm[:]), psG[:], mask, OP.mult)
        psM = pp.tile([128, 128], FP32, name=f"psM_{idx}", tag="b5",
                      padded_shape=[128, L])
        mmr(psM[:], Pm[:], phi)
        Ms = sb.tile([128, 128], FP32, name=f"Ms_{idx}", tag="Ms")
        nc.scalar.activation(r(Ms[:]), psM[:], AF.Copy)
        # psG2 = phi^T Ms + I  (identity accumulated on PE) -> masked Gf
        # then psXG = Gf^T cur = xG + cur  (residual folded in)
        psG2 = pp.tile([128, 128], FP32, name=f"psG2_{idx}", tag="b6",
                       padded_shape=[128, L])
        mmr(psG2[:], phi, Ms[:], start=True, stop=False)
        mmr(psG2[:], ident, ident, start=False, stop=True)
        Gf = sb.tile([128, 128], FP32, name=f"Gf_{idx}", tag="Gf")
        nc.vector.tensor_tensor(r(Gf[:]), psG2[:], mask, OP.mult)
        psXG = pp.tile([128, L], FP32, name=f"psXG_{idx}", tag="b7")
        mmr(psXG[:], Gf[:], cur[:])
        w1 = sb.tile([128, L], FP32, name=f"w1_{idx}", tag="w1")
        nc.vector.tensor_tensor(w1[:], sbc_s[:], U[:], OP.mult)
        nxt = sb.tile([128, L], FP32, name=f"nxt_{idx}", tag="nxt", bufs=2)
        nc.vector.tensor_tensor(r(nxt[:]), psXG[:], w1[:], OP.subtract)
        return nxt

    # ---- network -----------------------------------------------------------
    st1 = bn_send(fc("fc1", X, plain=True), "fc1", "bn1")
    h1n = bn_recv(st1, "bn1", 96.0, "bn1")
    st2 = bn_send(fc("fc2", h1n), "fc2", "bn2")
    h2n = bn_recv(st2, "bn2", 192.0, "bn2")
    ps3 = fc("fc3", h2n)
    # sigmoid(z+b) = 0.5 + 0.5*tanh(0.5 z + 0.5 b); rows c>=12 zeroed by colm
    th = sb.tile([128, L], FP32, name="th")
    nc.scalar.activation(th[:], ps3[:], AF.Tanh, scale=0.5, bias=Bv("fc3"))
    enc1 = sb.tile([128, L], FP32, name="enc1")
    nc.vector.tensor_scalar_mul(enc1[:], th[:], colm)
    enc = sb.tile([128, L], FP32, name="enc")
    nc.vector.tensor_scalar_add(r(enc[:]), enc1[:], colm)
    # dummy Rsq so the table swap back from the tanh set happens now,
    # hidden under relmod1, instead of on the bn4 critical path
    dums = sb.tile([1, 1], FP32, name="dums")
    nc.scalar.activation(dums[:], warm[:], AF.Abs_reciprocal_sqrt)

    cur = enc
    for i in range(4):
        cur = relmod(cur, f"u{i + 1}", i)

    st4 = bn_send(fc("fc4", cur), "fc4", "bn4")
    h4n = bn_recv(st4, "bn4", 96.0, "bn4")
    ps5 = fc("fc5", h4n)
    h5 = sb.tile([128, L], FP32, name="h5")
    nc.scalar.activation(r(h5[:]), ps5[:], AF.Relu, bias=Bv("fc5"))
    psOT = pp.tile([128, 128], FP32, name="psOT", tag="b1",
                   padded_shape=[128, L])
    for j in range(4):
        mmr(psOT[:, 32 * j:32 * (j + 1)], h5[:, 128 * j:128 * (j + 1)],
            PA2r[:, C_R67:C_R67 + 32])
    outs = sb.tile([128, 128], FP32, name="outs")
    nc.scalar.activation(outs[:], psOT[:], AF.Copy)
    nc.sync.dma_start(yp_d[:, :], outs[:])


# ---- host-side packing -----------------------------------------------------

def _pack_params(inputs):
    f32 = np.float32
    pa1 = np.zeros((128, KA1), f32)
    pa2 = np.zeros((128, KA2), f32)
    pb = np.zeros((128, 128), f32)
    pbn = np.zeros((12, 2 * L), f32)

    def spread(w):
        o, i = w.shape
        blk = np.zeros((GS, GS), f32)
        blk[:i, :o] = np.asarray(w, f32).T
        bd = np.zeros((128, 128), f32)
        for g in range(NG):
            bd[GS * g:GS * g + GS, GS * g:GS * g + GS] = blk
        return bd

    def bias_col(b, extra_one=False):
        col = np.zeros(GS, f32)
        col[:len(b)] = np.asarray(b, f32)
        if extra_one:
            col[3] = 1.0
        return np.tile(col, NG)

    a_r = [float(inputs[f"wr{i}"][0] * inputs[f"ps{i}"][0]
                 * inputs[f"ph{i}"][0]) / N for i in (1, 2, 3, 4)]

    pa1[:, A_BIAS + 0] = bias_col(inputs["fc1_b"])
    pa1[:, A_BIAS + 1] = bias_col(inputs["fc2_b"])
    pa1[:, A_BIAS + 2] = bias_col(0.5 * np.asarray(inputs["fc3_b"], f32))
    for i in (1, 2, 3, 4):
        pa1[:, A_BIAS + 2 + i] = bias_col(a_r[i - 1]
                                          * np.asarray(inputs[f"u{i}_b"], f32))
    pa1[:, A_BIAS + 7] = bias_col(inputs["fc4_b"])
    pa1[:, A_BIAS + 8] = bias_col(inputs["fc5_b"], extra_one=True)

    gidx = np.arange(128) // GS
    for j in range(4):
        pa1[:, A_OF + j] = (gidx % 4 == j).astype(f32)
    for g in range(NG):
        pa1[:, A_OC16 + g] = (gidx == g).astype(f32)
    pa1[:, A_COLM] = 0.5 * (np.arange(128) % GS < D4).astype(f32)
    for j, d in enumerate((96.0, 192.0, 96.0)):
        pa1[:, A_EPS + j] = EPS * d * d
    pa1[:, A_W1:A_W1 + 128] = spread(inputs["fc1_w"])

    for j in range(4):
        pa2[j, C_BC4R:C_BC4R + 128] = (gidx % 4 == j).astype(f32)
    for g in range(NG):
        pa2[g, C_BC8R:C_BC8R + 128] = (gidx == g).astype(f32)
    blk16 = np.ones((GS, GS), f32)
    for g in range(NG):
        pa2[GS * g:GS * g + GS, C_MASK + GS * g:C_MASK + GS * g + GS] = blk16
    pa2[:, C_ID:C_ID + 128] = np.eye(128, dtype=f32)
    for k, nm in (("fc2", "fc2_w"), ("fc3", "fc3_w"), ("fc4", "fc4_w"),
                  ("fc5", "fc5_w")):
        pa2[:, W_OFF[k]:W_OFF[k] + 128] = spread(inputs[nm])
    for i in (1, 2, 3, 4):
        pa2[:, W_OFF[f"u{i}"]:W_OFF[f"u{i}"] + 128] = spread(
            a_r[i - 1] * np.asarray(inputs[f"u{i}_w"], f32))
    w67 = np.zeros((4, GS), f32)
    w67[0, :F] = np.asarray(inputs["fc6_w"], f32)[0]
    w67[1:3, :F] = np.asarray(inputs["fc7_w"], f32)
    w67[0, 3] = float(inputs["fc6_b"][0])
    w67[1, 3] = float(inputs["fc7_b"][0])
    w67[2, 3] = float(inputs["fc7_b"][1])
    for g in range(NG):
        pa2[GS * g:GS * g + GS, C_R67 + 4 * g:C_R67 + 4 * g + 4] = w67.T

    phi = np.zeros((128, 128), f32)
    ci = np.arange(128) % GS
    for p in range(128):
        phi[p, :] = ((gidx // 4 == gidx[p] // 4) & (ci == ci[p])).astype(f32)
    pb[:, :] = phi
    for row, gn, bn, d in ((0, "bn1_g", "bn1_b", 96.0),
                           (4, "bn2_g", "bn2_b", 192.0),
                           (8, "bn4_g", "bn4_b", 96.0)):
        pbn[row:row + 4, 0:L] = d * np.asarray(inputs[gn], f32).reshape(4, L)
        pbn[row:row + 4, L:2 * L] = np.asarray(inputs[bn], f32).reshape(4, L)
    return pa1, pa2, pb, pbn


def _pack_x(x_core):
    f32 = np.float32
    xp = np.zeros((128, L), f32)
    # xp[16*(4b+j)+c, t] = x[b, 512j+t, c]
    xr = np.asarray(x_core, f32).reshape(BPC, 4, L, F).transpose(0, 1, 3, 2)
    xp.reshape(NG, GS, L)[:, :F, :] = xr.reshape(NG, F, L)
    return xp


def _unpack_y(yp):
    f32 = np.float32
    yv = np.asarray(yp, f32).reshape(128, 4, NG, 4)[:, :, :, :F]  # [t,ch,g,o]
    og = yv.transpose(2, 1, 0, 3)                                 # [g,ch,t,o]
    return np.ascontiguousarray(og.reshape(BPC, 4 * 4 * 128, F))


_PROGRAM = None


def _get_program():
    global _PROGRAM
    if _PROGRAM is None:
        _PROGRAM = _build()
    return _PROGRAM


def run(inputs, trace=False, **kw):
    inputs = {k: np.ascontiguousarray(np.asarray(v, np.float32))
              for k, v in inputs.items()}
    nc = _get_program()
    pa1, pa2, pb, pbn = _pack_params(inputs)
    in_maps = []
    for i in range(NCORES):
        in_maps.append({
            "pa1": pa1, "pa2": pa2, "pb": pb, "pbn": pbn,
            "xp": _pack_x(inputs["x"][BPC * i:BPC * (i + 1)]),
        })
    last_exc = None
    for attempt in range(3):
        try:
            res = run_bass_kernel_spmd(
                nc, in_maps, core_ids=list(range(NCORES)), trace=trace, **kw)
            break
        except Exception as e:  # transient NRT_EXEC_UNIT_UNRECOVERABLE flakes
            last_exc = e
            import time
            time.sleep(5)
    else:
        raise last_exc
    out = np.concatenate([_unpack_y(res.results[i]["yp"])
                          for i in range(NCORES)], axis=0)
    return out, res


def kernel(**inputs) -> np.ndarray:
    out, _ = run(inputs)
    return out


# revision 3
# speedup vs baseline: 1.3161x; 1.0109x over previous
"""Trainium2 Bass kernel v2 for nn_Generator_34127810134219 (gnn_message_passing).

Strategy vs v1:
- ALL parameters and constant patterns (block-diagonal weights, masks, phi,
  identity, fold selectors, bn vectors) are packed on the HOST into one
  [128, K] f32 array, loaded as three tiles with overlapped DMAs.
- x is host-packed into the on-chip channel-major layout [128, 512] (one
  contiguous DMA); the output is written token-major to a [128,128] scratch
  (one DMA) and unpacked on host.
- relmod scale a_r = wr*ps*ph/N is folded into the u-weights/biases on host
  (relu(a*z) = a*relu(z), a>0).
- the residual "+cur" of each relmod is folded into the Gram apply by
  accumulating the identity onto the spread Gram matrix (PSUM accumulation),
  so the tail is a single subtract.
- ACT stays on the abs_reciprocal_sqrt table set; the one Tanh (sigmoid via
  0.5+0.5*tanh(z/2)) triggers table swaps that are hidden inside collective
  windows (a dummy Rsq right after the tanh swaps back early).
- bn statistics use AllReduce on raw sums [8,512]; the recv side works on
  sums directly: rstd' = 1/sqrt(q*D - s^2 + eps*D^2), a = (g*D)*rstd',
  shift = a*s/D - beta, saving the mean division.
"""

import numpy as np

import concourse.bass as bass
import concourse.bacc as bacc
import concourse.tile as tile
import concourse.mybir as mybir
from concourse.bass_utils import run_bass_kernel_spmd

FP32 = mybir.dt.float32
F32R = mybir.dt.float32r
AF = mybir.ActivationFunctionType
OP = mybir.AluOpType

B, N, F = 16, 2048, 3
D2, D4 = 6, 12
NCORES = 8
BPC = B // NCORES
NG = 8
L = 512
GS = 16
EPS = 1e-5

# ---- packed-constant column maps (three tiles) -----------------------------
# tile A1 [128, KA1]: needed by fc1/bn1-send
A_BIAS = 0      # 10 cols: fc1, fc2, 0.5*fc3, u1..u4 (a-scaled), fc4, fc5(+1@c3), spare
A_OF = 10       # 4: onesfold[(g,c), j] = [g%4==j]
A_OC16 = 14     # 8: ones_c16[(g,c), g'] = [g==g']
A_COLM = 22     # 1: 0.5*colmask12
A_EPS = 23      # 3: eps*D^2 for bn1, bn2, bn4
A_W1 = 26       # 128: fc1 block-diag
KA1 = 154
# tile A2 [128, KA2]
C_BC4R = 0      # 128 (partitions 0:4): bc4r[j,(g,c)] = [g%4==j]
C_BC8R = 128    # 128 (partitions 0:8): bc8r[j,(g,c)] = [g==j]
C_MASK = 256    # 128: mask_diag
C_ID = 384      # 128: identity
C_W2 = 512      # 8x128: fc2, fc3, u1, u2, u3, u4, fc4, fc5
C_R67 = 1536    # 32: token-major fc6+fc7 rhs (with bias row)
KA2 = 1568
# tile B: phi [128,128]; per-bn vectors [4, 1024] = (g*D | beta)

W_OFF = {"fc2": C_W2, "fc3": C_W2 + 128, "u1": C_W2 + 256,
         "u2": C_W2 + 384, "u3": C_W2 + 512, "u4": C_W2 + 640,
         "fc4": C_W2 + 768, "fc5": C_W2 + 896}
B_IDX = {"fc1": 0, "fc2": 1, "fc3": 2, "u1": 3, "u2": 4, "u3": 5, "u4": 6,
         "fc4": 7, "fc5": 8}
BN_ROW = {"bn1": 0, "bn2": 4, "bn4": 8}
BN_EPS = {"bn1": 0, "bn2": 1, "bn4": 2}


def _build(single_core=False):
    nc = bacc.Bacc(
        "TRN2",
        target_bir_lowering=False,
        debug=False,
        enable_asserts=False,
        num_devices=1 if single_core else NCORES,
    )

    xp_d = nc.dram_tensor("xp", [128, L], FP32, kind="ExternalInput")
    pa1_d = nc.dram_tensor("pa1", [128, KA1], FP32, kind="ExternalInput")
    pa2_d = nc.dram_tensor("pa2", [128, KA2], FP32, kind="ExternalInput")
    pb_d = nc.dram_tensor("pb", [128, 128], FP32, kind="ExternalInput")
    pbn_d = nc.dram_tensor("pbn", [12, 2 * L], FP32, kind="ExternalInput")
    yp_d = nc.dram_tensor("yp", [128, 128], FP32, kind="ExternalOutput")

    with tile.TileContext(nc) as tc:
        with (
            tc.tile_pool(name="consts", bufs=1) as cp,
            tc.tile_pool(name="sb", bufs=1) as sb,
            tc.tile_pool(name="pp", bufs=1, space="PSUM") as pp,
            tc.tile_pool(name="dram", bufs=1, space="DRAM") as dr,
        ):
            _emit(nc, tc, cp, sb, pp, dr, xp_d, pa1_d, pa2_d, pb_d, pbn_d,
                  yp_d, single_core=single_core)

    nc.compile()
    return nc


def _emit(nc, tc, cp, sb, pp, dr, xp_d, pa1_d, pa2_d, pb_d, pbn_d, yp_d,
          single_core=False):
    def mmr(out, lhsT, rhs, **kw):
        nc.tensor.matmul(out, lhsT.bitcast(F32R), rhs.bitcast(F32R), **kw)

    def r(ap):
        return ap.bitcast(F32R)

    # ---- input DMAs (xp most urgent, then A1, A2 on sync; B on SWDGE) ------
    X = sb.tile([128, L], FP32, name="X")
    nc.sync.dma_start(X[:], xp_d[:, :])
    PA1 = cp.tile([128, KA1], FP32, name="PA1")
    nc.sync.dma_start(PA1[:], pa1_d[:, :])
    PA2 = cp.tile([128, KA2], FP32, name="PA2")
    nc.sync.dma_start(PA2[:], pa2_d[:, :])
    PB = cp.tile([128, 128], FP32, name="PB")
    nc.sync.dma_start(PB[:], pb_d[:, :])
    BNV = {}
    for bi, bk in enumerate(("bn1", "bn2", "bn4")):
        BNV[bk] = cp.tile([4, 2 * L], FP32, name=f"bnv_{bk}")
        nc.sync.dma_start(BNV[bk][:], pbn_d[4 * bi:4 * bi + 4, :])

    # ACT table warmup: resolve the abs_reciprocal_sqrt set up front.
    warm = sb.tile([1, 1], FP32, name="warm")
    nc.vector.memset(warm[:], 1.0)
    warmo = sb.tile([1, 1], FP32, name="warmo")
    nc.scalar.activation(warmo[:], warm[:], AF.Abs_reciprocal_sqrt)

    # f32r-rounded copies of every matmul-feeding constant region
    # (walrus requires f32r matmul operands to have f32r producers)
    selr = cp.tile([128, 12], FP32, name="selr")
    nc.vector.tensor_copy(selr[:].bitcast(F32R), PA1[:, A_OF:A_OF + 12])
    W1r = cp.tile([128, 128], FP32, name="W1r")
    nc.vector.tensor_copy(W1r[:].bitcast(F32R), PA1[:, A_W1:A_W1 + 128])
    Xr = sb.tile([128, L], FP32, name="Xr")
    nc.vector.tensor_copy(Xr[:].bitcast(F32R), X[:])
    PA2r = cp.tile([128, KA2], FP32, name="PA2r")
    nc.vector.tensor_copy(PA2r[:].bitcast(F32R), PA2[:])
    phir = cp.tile([128, 128], FP32, name="phir")
    nc.vector.tensor_copy(phir[:].bitcast(F32R), PB[:])

    def Wv(k):
        if k == "fc1":
            return W1r[:]
        return PA2r[:, W_OFF[k]:W_OFF[k] + 128]

    def Bv(k):
        j = B_IDX[k]
        return PA1[:, A_BIAS + j:A_BIAS + j + 1]

    mask = PA2r[:, C_MASK:C_MASK + 128]
    phi = phir[:]
    ident = PA2r[:, C_ID:C_ID + 128]
    bc4r = PA2r[0:4, C_BC4R:C_BC4R + 128]
    bc8r = PA2r[0:NG, C_BC8R:C_BC8R + 128]
    onesfold = selr[:, 0:4]
    oc16 = selr[:, 4:12]
    colm = PA1[:, A_COLM:A_COLM + 1]

    def fc(k, src, plain=False):
        ps = pp.tile([128, L], FP32, name=f"ps_{k}", tag="b0")
        if plain:
            nc.tensor.matmul(ps[:], Wv(k), src[:])
        else:
            mmr(ps[:], Wv(k), src[:])
        return ps

    # ---- batchnorm ---------------------------------------------------------
    def bn_send(h_ps, k, tag):
        hs = sb.tile([128, L], FP32, name=f"hs_{tag}")
        nc.scalar.add(r(hs[:]), h_ps[:], Bv(k))
        sq = sb.tile([128, L], FP32, name=f"sq_{tag}")
        nc.scalar.activation(r(sq[:]), h_ps[:], AF.Square, bias=Bv(k))
        pk_s = pp.tile([4, L], FP32, name=f"pks_{tag}", tag="b1",
                       padded_shape=[128, L])
        pk_q = pp.tile([4, L], FP32, name=f"pkq_{tag}", tag="b2",
                       padded_shape=[128, L])
        mmr(pk_s[:], onesfold, hs[:])
        mmr(pk_q[:], onesfold, sq[:])
        sk = sb.tile([4, 2 * L], FP32, name=f"sk_{tag}")
        nc.scalar.activation(sk[:, 0:L], pk_s[:], AF.Copy)
        nc.vector.tensor_copy(sk[:, L:2 * L], pk_q[:])
        cc_in = dr.tile([8, L], FP32, name=f"ccin_{tag}")
        cc_out = dr.tile([8, L], FP32, name=f"ccout_{tag}")
        nc.sync.dma_start(cc_in[:].rearrange("(r p) t -> p r t", p=4),
                          sk[:].rearrange("p (r t) -> p r t", r=2))
        if single_core:
            # stand-in for the 8-core AllReduce: 3 dummy DMAs + the real
            # cc_out write, serialized on the HWDGE, model ~4-5us latency
            cc_mid = dr.tile([24, L], FP32, name=f"ccmid_{tag}")
            for rr in range(3):
                nc.sync.dma_start(cc_mid[8 * rr:8 * rr + 8, :], cc_in[:])
            nc.sync.dma_start(cc_out[:], cc_in[:])
        else:
            nc.gpsimd.collective_compute(
                "AllReduce",
                OP.add,
                replica_groups=[list(range(NCORES))],
                ins=[cc_in.opt()],
                outs=[cc_out.opt()],
            )
        return hs, cc_out

    def bn_recv(state, k, denom, tag):
        """From AllReduced sums s=rows 0:4, q=rows 4:8 of cc_out:
        rstd' = 1/sqrt(q*D - s^2 + eps*D^2)  (= rstd/D)
        a = (g*D)*rstd'; shift = a*s/D - beta; hn = relu(a*hs - shift)."""
        hs, cc_out = state
        ecol = PA1[0:4, A_EPS + BN_EPS[k]:A_EPS + BN_EPS[k] + 1]
        gath = sb.tile([4, 2 * L], FP32, name=f"gath_{tag}")
        nc.sync.dma_start(gath[:].rearrange("p (r t) -> p r t", r=2),
                          cc_out[:].rearrange("(r p) t -> p r t", p=4))
        msq = sb.tile([4, L], FP32, name=f"msq_{tag}")
        nc.vector.tensor_tensor(msq[:], gath[:, 0:L], gath[:, 0:L], OP.mult)
        varp = sb.tile([4, L], FP32, name=f"varp_{tag}")
        nc.vector.scalar_tensor_tensor(varp[:], gath[:, L:2 * L], denom,
                                       msq[:], OP.mult, OP.subtract)
        rstd = sb.tile([4, L], FP32, name=f"rstd_{tag}")
        nc.scalar.activation(rstd[:], varp[:], AF.Abs_reciprocal_sqrt,
                             bias=ecol)
        a = sb.tile([4, L], FP32, name=f"a_{tag}")
        nc.vector.tensor_tensor(r(a[:]), rstd[:], BNV[k][:, 0:L], OP.mult)
        am = sb.tile([4, L], FP32, name=f"am_{tag}")
        nc.vector.tensor_tensor(am[:], a[:], gath[:, 0:L], OP.mult)
        negd = sb.tile([4, L], FP32, name=f"negd_{tag}")
        nc.vector.scalar_tensor_tensor(r(negd[:]), am[:], 1.0 / denom,
                                       BNV[k][:, L:2 * L],
                                       OP.mult, OP.subtract)
        A_bc = pp.tile([128, L], FP32, name=f"Abc_{tag}", tag="b2")
        mmr(A_bc[:], bc4r, a[:])
        nD_bc = pp.tile([128, L], FP32, name=f"nDbc_{tag}", tag="b3")
        mmr(nD_bc[:], bc4r, negd[:])
        t2 = sb.tile([128, L], FP32, name=f"t2_{tag}")
        nc.vector.tensor_tensor(t2[:], hs[:], A_bc[:], OP.mult)
        t3 = sb.tile([128, L], FP32, name=f"t3_{tag}")
        nc.vector.tensor_tensor(t3[:], t2[:], nD_bc[:], OP.subtract)
        hn = sb.tile([128, L], FP32, name=f"hn_{tag}")
        nc.vector.tensor_relu(r(hn[:]), t3[:])
        return hn

    # ---- relation module ---------------------------------------------------
    def relmod(cur, k, idx):
        # PE: psU, pTc, pTu, psS, psG, Sbc, psM, psG2(+I), psXG
        psU = pp.tile([128, L], FP32, name=f"psU_{idx}", tag="b0")
        mmr(psU[:], Wv(k), cur[:])
        pTc = pp.tile([128, 4 * 128], FP32, name=f"pTc_{idx}", tag="b1")
        for j in range(4):
            nc.tensor.transpose(pTc[:, 128 * j:128 * (j + 1)],
                                cur[:, 128 * j:128 * (j + 1)], ident)
        U = sb.tile([128, L], FP32, name=f"U_{idx}", tag="U")
        nc.scalar.activation(r(U[:]), psU[:], AF.Relu, bias=Bv(k))
        sq = sb.tile([128, L], FP32, name=f"rsq_{idx}", tag="rsq")
        nc.scalar.activation(r(sq[:]), cur[:], AF.Square)
        curT = sb.tile([128, 4 * 128], FP32, name=f"curT_{idx}", tag="curT")
        nc.vector.tensor_copy(r(curT[:]), pTc[:])
        pTu = pp.tile([128, 4 * 128], FP32, name=f"pTu_{idx}", tag="b2")
        for j in range(4):
            nc.tensor.transpose(pTu[:, 128 * j:128 * (j + 1)],
                                U[:, 128 * j:128 * (j + 1)], ident)
        UT = sb.tile([128, 4 * 128], FP32, name=f"UT_{idx}", tag="UT")
        nc.vector.tensor_copy(r(UT[:]), pTu[:])
        psS = pp.tile([NG, L], FP32, name=f"psS_{idx}", tag="b3",
                      padded_shape=[128, L])
        mmr(psS[:], oc16, sq[:])
        psG = pp.tile([128, 128], FP32, name=f"psG_{idx}", tag="b4",
                      padded_shape=[128, L])
        for j in range(4):
            mmr(psG[:], UT[:, 128 * j:128 * (j + 1)],
                curT[:, 128 * j:128 * (j + 1)],
                start=(j == 0), stop=(j == 3))
        sS = sb.tile([NG, L], FP32, name=f"sS_{idx}", tag="sS")
        nc.vector.tensor_copy(r(sS[:]), psS[:])
        Sbc = pp.tile([128, L], FP32, name=f"Sbc_{idx}", tag="b1")
        mmr(Sbc[:], bc8r, sS[:])
        sbc_s = sb.tile([128, L], FP32, name=f"sbcs_{idx}", tag="sbcs")
        nc.scalar.activation(sbc_s[:], Sbc[:], AF.Copy)
        Pm = sb.tile([128, 128], FP32, name=f"Pm_{idx}", tag="Pm")
        nc.vector.tensor_tensor(r(Pm[:]), psG[:], mask, OP.mult)
        psM = pp.tile([128, 128], FP32, name=f"psM_{idx}", tag="b5",
                      padded_shape=[128, L])
        mmr(psM[:], Pm[:], phi)
        Ms = sb.tile([128, 128], FP32, name=f"Ms_{idx}", tag="Ms")
        nc.scalar.activation(r(Ms[:]), psM[:], AF.Copy)
        # psG2 = phi^T Ms + I  (identity accumulated on PE) -> masked Gf
        # then psXG = Gf^T cur = xG + cur  (residual folded in)
        psG2 = pp.tile([128, 128], FP32, name=f"psG2_{idx}", tag="b6",
                       padded_shape=[128, L])
        mmr(psG2[:], phi, Ms[:], start=True, stop=False)
        mmr(psG2[:], ident, ident, start=False, stop=True)
        Gf = sb.tile([128, 128], FP32, name=f"Gf_{idx}", tag="Gf")
        nc.vector.tensor_tensor(r(Gf[:]), psG2[:], mask, OP.mult)
        psXG = pp.tile([128, L], FP32, name=f"psXG_{idx}", tag="b7")
        mmr(psXG[:], Gf[:], cur[:])
        w1 = sb.tile([128, L], FP32, name=f"w1_{idx}", tag="w1")
        nc.vector.tensor_tensor(w1[:], sbc_s[:], U[:], OP.mult)
        nxt = sb.tile([128, L], FP32, name=f"nxt_{idx}", tag="nxt", bufs=2)
        nc.vector.tensor_tensor(r(nxt[:]), psXG[:], w1[:], OP.subtract)
        return nxt

    # ---- network -----------------------------------------------------------
    st1 = bn_send(fc("fc1", Xr), "fc1", "bn1")
    h1n = bn_recv(st1, "bn1", 96.0, "bn1")
    st2 = bn_send(fc("fc2", h1n), "fc2", "bn2")
    h2n = bn_recv(st2, "bn2", 192.0, "bn2")
    ps3 = fc("fc3", h2n)
    # sigmoid(z+b) = 0.5 + 0.5*tanh(0.5 z + 0.5 b); rows c>=12 zeroed by colm
    th = sb.tile([128, L], FP32, name="th")
    nc.scalar.activation(th[:], ps3[:], AF.Tanh, scale=0.5, bias=Bv("fc3"))
    enc1 = sb.tile([128, L], FP32, name="enc1")
    nc.vector.tensor_scalar_mul(enc1[:], th[:], colm)
    enc = sb.tile([128, L], FP32, name="enc")
    nc.vector.tensor_scalar_add(r(enc[:]), enc1[:], colm)
    # dummy Rsq so the table swap back from the tanh set happens now,
    # hidden under relmod1, instead of on the bn4 critical path
    dums = sb.tile([1, 1], FP32, name="dums")
    nc.scalar.activation(dums[:], warm[:], AF.Abs_reciprocal_sqrt)

    cur = enc
    for i in range(4):
        cur = relmod(cur, f"u{i + 1}", i)

    st4 = bn_send(fc("fc4", cur), "fc4", "bn4")
    h4n = bn_recv(st4, "bn4", 96.0, "bn4")
    ps5 = fc("fc5", h4n)
    h5 = sb.tile([128, L], FP32, name="h5")
    nc.scalar.activation(r(h5[:]), ps5[:], AF.Relu, bias=Bv("fc5"))
    psOT = pp.tile([128, 128], FP32, name="psOT", tag="b1",
                   padded_shape=[128, L])
    for j in range(4):
        mmr(psOT[:, 32 * j:32 * (j + 1)], h5[:, 128 * j:128 * (j + 1)],
            PA2r[:, C_R67:C_R67 + 32])
    outs = sb.tile([128, 128], FP32, name="outs")
    nc.scalar.activation(outs[:], psOT[:], AF.Copy)
    nc.sync.dma_start(yp_d[:, :], outs[:])


# ---- host-side packing -----------------------------------------------------

def _pack_params(inputs):
    f32 = np.float32
    pa1 = np.zeros((128, KA1), f32)
    pa2 = np.zeros((128, KA2), f32)
    pb = np.zeros((128, 128), f32)
    pbn = np.zeros((12, 2 * L), f32)

    def spread(w):
        o, i = w.shape
        blk = np.zeros((GS, GS), f32)
        blk[:i, :o] = np.asarray(w, f32).T
        bd = np.zeros((128, 128), f32)
        for g in range(NG):
            bd[GS * g:GS * g + GS, GS * g:GS * g + GS] = blk
        return bd

    def bias_col(b, extra_one=False):
        col = np.zeros(GS, f32)
        col[:len(b)] = np.asarray(b, f32)
        if extra_one:
            col[3] = 1.0
        return np.tile(col, NG)

    a_r = [float(inputs[f"wr{i}"][0] * inputs[f"ps{i}"][0]
                 * inputs[f"ph{i}"][0]) / N for i in (1, 2, 3, 4)]

    pa1[:, A_BIAS + 0] = bias_col(inputs["fc1_b"])
    pa1[:, A_BIAS + 1] = bias_col(inputs["fc2_b"])
    pa1[:, A_BIAS + 2] = bias_col(0.5 * np.asarray(inputs["fc3_b"], f32))
    for i in (1, 2, 3, 4):
        pa1[:, A_BIAS + 2 + i] = bias_col(a_r[i - 1]
                                          * np.asarray(inputs[f"u{i}_b"], f32))
    pa1[:, A_BIAS + 7] = bias_col(inputs["fc4_b"])
    pa1[:, A_BIAS + 8] = bias_col(inputs["fc5_b"], extra_one=True)

    gidx = np.arange(128) // GS
    for j in range(4):
        pa1[:, A_OF + j] = (gidx % 4 == j).astype(f32)
    for g in range(NG):
        pa1[:, A_OC16 + g] = (gidx == g).astype(f32)
    pa1[:, A_COLM] = 0.5 * (np.arange(128) % GS < D4).astype(f32)
    for j, d in enumerate((96.0, 192.0, 96.0)):
        pa1[:, A_EPS + j] = EPS * d * d
    pa1[:, A_W1:A_W1 + 128] = spread(inputs["fc1_w"])

    for j in range(4):
        pa2[j, C_BC4R:C_BC4R + 128] = (gidx % 4 == j).astype(f32)
    for g in range(NG):
        pa2[g, C_BC8R:C_BC8R + 128] = (gidx == g).astype(f32)
    blk16 = np.ones((GS, GS), f32)
    for g in range(NG):
        pa2[GS * g:GS * g + GS, C_MASK + GS * g:C_MASK + GS * g + GS] = blk16
    pa2[:, C_ID:C_ID + 128] = np.eye(128, dtype=f32)
    for k, nm in (("fc2", "fc2_w"), ("fc3", "fc3_w"), ("fc4", "fc4_w"),
                  ("fc5", "fc5_w")):
        pa2[:, W_OFF[k]:W_OFF[k] + 128] = spread(inputs[nm])
    for i in (1, 2, 3, 4):
        pa2[:, W_OFF[f"u{i}"]:W_OFF[f"u{i}"] + 128] = spread(
            a_r[i - 1] * np.asarray(inputs[f"u{i}_w"], f32))
    w67 = np.zeros((4, GS), f32)
    w67[0, :F] = np.asarray(inputs["fc6_w"], f32)[0]
    w67[1:3, :F] = np.asarray(inputs["fc7_w"], f32)
    w67[0, 3] = float(inputs["fc6_b"][0])
    w67[1, 3] = float(inputs["fc7_b"][0])
    w67[2, 3] = float(inputs["fc7_b"][1])
    for g in range(NG):
        pa2[GS * g:GS * g + GS, C_R67 + 4 * g:C_R67 + 4 * g + 4] = w67.T

    phi = np.zeros((128, 128), f32)
    ci = np.arange(128) % GS
    for p in range(128):
        phi[p, :] = ((gidx // 4 == gidx[p] // 4) & (ci == ci[p])).astype(f32)
    pb[:, :] = phi
    for row, gn, bn, d in ((0, "bn1_g", "bn1_b", 96.0),
                           (4, "bn2_g", "bn2_b", 192.0),
                           (8, "bn4_g", "bn4_b", 96.0)):
        pbn[row:row + 4, 0:L] = d * np.asarray(inputs[gn], f32).reshape(4, L)
        pbn[row:row + 4, L:2 * L] = np.asarray(inputs[bn], f32).reshape(4, L)
    return pa1, pa2, pb, pbn


def _pack_x(x_core):
    f32 = np.float32
    xp = np.zeros((128, L), f32)
    # xp[16*(4b+j)+c, t] = x[b, 512j+t, c]
    xr = np.asarray(x_core, f32).reshape(BPC, 4, L, F).transpose(0, 1, 3, 2)
    xp.reshape(NG, GS, L)[:, :F, :] = xr.reshape(NG, F, L)
    return xp


def _unpack_y(yp):
    f32 = np.float32
    yv = np.asarray(yp, f32).reshape(128, 4, NG, 4)[:, :, :, :F]  # [t,ch,g,o]
    og = yv.transpose(2, 1, 0, 3)                                 # [g,ch,t,o]
    return np.ascontiguousarray(og.reshape(BPC, 4 * 4 * 128, F))


_PROGRAM = None


def _get_program():
    global _PROGRAM
    if _PROGRAM is None:
        _PROGRAM = _build()
    return _PROGRAM


def run(inputs, trace=False, **kw):
    inputs = {k: np.ascontiguousarray(np.asarray(v, np.float32))
              for k, v in inputs.items()}
    nc = _get_program()
    pa1, pa2, pb, pbn = _pack_params(inputs)
    in_maps = []
    for i in range(NCORES):
        in_maps.append({
            "pa1": pa1, "pa2": pa2, "pb": pb, "pbn": pbn,
            "xp": _pack_x(inputs["x"][BPC * i:BPC * (i + 1)]),
        })
    last_exc = None
    for attempt in range(3):
        try:
            res = run_bass_kernel_spmd(
                nc, in_maps, core_ids=list(range(NCORES)), trace=trace, **kw)
            break
        except Exception as e:  # transient NRT_EXEC_UNIT_UNRECOVERABLE flakes
            last_exc = e
            import time
            time.sleep(5)
    else:
        raise last_exc
    out = np.concatenate([_unpack_y(res.results[i]["yp"])
                          for i in range(NCORES)], axis=0)
    return out, res


def kernel(**inputs) -> np.ndarray:
    out, _ = run(inputs)
    return out
